# revision 33
# baseline (speedup 1.0000x reference)
"""Trainium2 Bass kernel for an audio-anomaly VQ-VAE forward pass.

Math (B=65536 rows, all weights shared):
  d  = df @ dp_w.T + dp_b                      [B, 512]
  t  = tf @ tp_w.T + tp_b                      [B, 512]
  (seq len is 1 so softmax==1 and MHA reduces to its value path)
  d2 = d + LN(t;n1) @ A1 + cA1  with A1 = (out_w@wv@diag(n1_w)).T host-folded
  t2 = t + LN(d;n2) @ A2 + cA2
  u  = d2 @ e_w1[:, :512].T + t2 @ e_w1[:, 512:].T + e_b1'   (biases folded)
  h  = relu(LN(u)*e_lnw + e_lnb) @ e_w2.T + e_b2             [B, 64]
  sneg_k = 2 h.c_k - |c_k|^2 ; idx = argmax_k sneg           (== argmin dist)
  loss = 1.25 * mean(|h - c_idx|^2) = 1.25 * mean(|h|^2 - sneg_idx)
  rec  = relu(c_idx @ d_w1.T + d_b1) @ d_w2.T + d_b2  -> 2048-row table + gather

Device (8 cores, batch data-parallel, 8192 rows/core):
  batch-major tiles of 128 rows; activations transposed on the PE for matmul
  lhsT; fp32r (FP22) matmuls; LN via bn_stats reading PSUM; per-512-chunk
  Max8/MaxIndex argmax with masked-sum index merge; decoder as a 2048x1024
  DRAM table gathered by indirect DMA.
Host:
  folds weights, shards inputs, and exactly re-solves the few rows whose
  top-2 score gap is below a threshold (fp22 noise floor) in float64.
"""

import numpy as np

import concourse.bass as bass
import concourse.mybir as mybir
import concourse.tile as tile
from concourse import bacc
from concourse.bass import ds, ts
from concourse.masks import make_identity
from concourse.tile import add_dep_helper

F32 = mybir.dt.float32
F32R = mybir.dt.float32r
U32 = mybir.dt.uint32

B = 65536
D = 48
T = 768
F = 512
L = 64
K = 2048
NCORES = 8
BC = B // NCORES          # rows per core
SUB = 128                 # rows per sub-block
NSUB_FULL = BC // SUB     # 64

AF = mybir.ActivationFunctionType
ALU = mybir.AluOpType
AX = mybir.AxisListType


def _r(ap):
    return ap.bitcast(F32R)


def build_program(nsub=NSUB_FULL, use_f32r=True, with_dec=True, with_gather=True,
                  with_deps=True, stage=9,
                  acts_bufs=3, mid_bufs=3, tr_bufs=1, small_bufs=3, recg_bufs=3):
    """Build the per-core Bass program."""
    nc = bacc.Bacc("TRN2", target_bir_lowering=False, debug=False,
                   num_devices=NCORES)

    mmcast = _r if use_f32r else (lambda ap: ap)
    wcast = _r if use_f32r else (lambda ap: ap)

    rows = nsub * SUB
    # inputs
    tf_d = nc.dram_tensor("tf", [rows, T], F32, kind="ExternalInput")
    df_d = nc.dram_tensor("df", [rows, D], F32, kind="ExternalInput")
    wt_d = nc.dram_tensor("wt", [T, F], F32, kind="ExternalInput")      # tp_w.T
    wd_d = nc.dram_tensor("wd", [D, F], F32, kind="ExternalInput")    # dp_w.T
    wa1_d = nc.dram_tensor("wa1", [F, F], F32, kind="ExternalInput")    # A1
    wa2_d = nc.dram_tensor("wa2", [F, F], F32, kind="ExternalInput")    # A2
    we1d_d = nc.dram_tensor("we1d", [F, F], F32, kind="ExternalInput")  # e_w1[:, :F].T
    we1t_d = nc.dram_tensor("we1t", [F, F], F32, kind="ExternalInput")  # e_w1[:, F:].T
    we2_d = nc.dram_tensor("we2", [F, L], F32, kind="ExternalInput")    # e_w2.T
    cbq_d = nc.dram_tensor("cbq", [L + 1, K], F32, kind="ExternalInput")  # [2cb.T; -|c|^2]
    cbt_d = nc.dram_tensor("cbt", [L, K], F32, kind="ExternalInput")    # codebook.T
    wd1_d = nc.dram_tensor("wd1", [L, F], F32, kind="ExternalInput")    # d_w1.T
    wd2_d = nc.dram_tensor("wd2", [F, 2 * F], F32, kind="ExternalInput")  # d_w2.T
    bt_d = nc.dram_tensor("bt", [1, F], F32, kind="ExternalInput")      # tp_b
    bd_d = nc.dram_tensor("bd", [1, F], F32, kind="ExternalInput")      # dp_b
    bu_d = nc.dram_tensor("bu", [1, F], F32, kind="ExternalInput")      # e_b1 folded
    bh_d = nc.dram_tensor("bh", [1, L], F32, kind="ExternalInput")      # e_b2
    bq1_d = nc.dram_tensor("bq1", [1, F], F32, kind="ExternalInput")    # d_b1
    bq2_d = nc.dram_tensor("bq2", [1, 2 * F], F32, kind="ExternalInput")  # d_b2
    lnw_d = nc.dram_tensor("lnw", [1, F], F32, kind="ExternalInput")    # e_lnw
    lnb_d = nc.dram_tensor("lnb", [1, F], F32, kind="ExternalInput")    # e_lnb

    # outputs
    rec_d = nc.dram_tensor("rec", [rows, 2 * F], F32, kind="ExternalOutput")
    m8v_d = nc.dram_tensor("m8v", [nsub, SUB, 8], F32, kind="ExternalOutput")
    m8i_d = nc.dram_tensor("m8i", [nsub, SUB], U32, kind="ExternalOutput")
    h2_d = nc.dram_tensor("h2", [nsub, SUB], F32, kind="ExternalOutput")

    # scratch
    dtab_d = nc.dram_tensor("dtab", [K, 2 * F], F32, kind="Internal")

    with tile.TileContext(nc) as tc:
        with (
            tc.tile_pool(name="singles", bufs=1) as singles,
            tc.tile_pool(name="small", bufs=small_bufs) as small,
            tc.tile_pool(name="recg", bufs=recg_bufs) as recp,
            tc.tile_pool(name="ps_mid", bufs=mid_bufs, space="PSUM") as ps_mid,
            tc.tile_pool(name="ps_tr", bufs=tr_bufs, space="PSUM") as ps_tr,
        ):
            # ---- constants ----
            ident = singles.tile([128, 128], F32)
            make_identity(nc, ident[:])
            ident_r = singles.tile([128, 128], F32)
            nc.scalar.activation(wcast(ident_r[:]), ident[:], AF.Copy)
            ones1 = singles.tile([1, 128], F32)
            nc.vector.memset(ones1[:], 1.0)
            ones_r = singles.tile([1, 128], F32)
            nc.scalar.activation(wcast(ones_r[:]), ones1[:], AF.Copy)
            eps_t = singles.tile([128, 1], F32)
            nc.vector.memset(eps_t[:], 1e-5)

            def ld(shape, dram, tag, rearr=False, cast=True):
                t_ = singles.tile(shape, F32, tag=tag)
                ap = dram.ap()
                if rearr:
                    ap = ap.rearrange("(c p) n -> p c n", p=128)
                c = wcast if cast else (lambda a: a)
                nc.sync.dma_start(c(t_[:]), c(ap))
                return t_

            wt_s = ld([128, T // 128, F], wt_d, "wt", rearr=True)
            wd_s = ld([D, F], wd_d, "wd")
            bd_s = ld([1, F], bd_d, "bd")
            wa1_s = ld([128, 4, F], wa1_d, "wa1", rearr=True)
            wa2_s = ld([128, 4, F], wa2_d, "wa2", rearr=True)
            we1d_s = ld([128, 4, F], we1d_d, "we1d", rearr=True)
            we1t_s = ld([128, 4, F], we1t_d, "we1t", rearr=True)
            we2_s = ld([128, 4, L], we2_d, "we2", rearr=True)
            cbq_s = ld([L + 1, K], cbq_d, "cbq")
            bt_s = ld([1, F], bt_d, "bt")
            bu_s = ld([1, F], bu_d, "bu")
            bh_s = ld([1, L], bh_d, "bh")
            lnw_s = ld([1, F], lnw_d, "lnw", cast=False)
            lnb_s = ld([1, F], lnb_d, "lnb", cast=False)

            def bias_mm(ps, bias_ap, n0, n1, start=False, stop=True):
                # += ones.T @ bias  (broadcast a [1, n] bias row over rows)
                nc.tensor.matmul(ps[:, n0:n1], mmcast(ones_r[:, :ps.shape[0]]),
                                 mmcast(bias_ap[:, n0:n1]),
                                 start=start, stop=stop)

            def transpose128(dst_sb, src_ap, pdim=128):
                # src [128, pdim] -> dst_sb [pdim, 128] via PE + ACT evict
                ps = ps_tr.tile([128, F], F32, tag="trb", bufs=tr_bufs)
                nc.tensor.transpose(out=wcast(ps[:pdim, :128]), in_=mmcast(src_ap),
                                    identity=wcast(ident_r[:]))
                nc.scalar.activation(wcast(dst_sb), ps[:pdim, :128], AF.Copy)

            def transpose_batch(dst_sb, src_sb, nchunk):
                # feature-major chunks via <=4-chunk PSUM batches, one ACT
                # evict per batch (each batch fits one PSUM bank)
                for b0 in range(0, nchunk, 4):
                    nb = min(4, nchunk - b0)
                    ps = ps_tr.tile([128, F], F32, tag="trb", bufs=tr_bufs)
                    for c in range(nb):
                        nc.tensor.transpose(
                            out=wcast(ps[:, ts(c, 128)]),
                            in_=mmcast(src_sb[:, ts(b0 + c, 128)]),
                            identity=wcast(ident_r[:]))
                    w = nb * 128
                    nc.scalar.activation(
                        wcast(dst_sb[:, b0 * 128:b0 * 128 + w]), ps[:, :w],
                        AF.Copy)

            # ---- decoder table (scoped pool; SBUF reused by acts after) ----
            dstores = []
            if with_dec:
                with tc.tile_pool(name="dec", bufs=2) as decp:
                    cbt_s = decp.tile([L, K], F32, tag="cbt")
                    nc.sync.dma_start(wcast(cbt_s[:]), wcast(cbt_d.ap()))
                    wd1_s = decp.tile([L, F], F32, tag="wd1")
                    nc.sync.dma_start(wcast(wd1_s[:]), wcast(wd1_d.ap()))
                    wd2_s = decp.tile([128, 4, 2 * F], F32, tag="wd2")
                    nc.sync.dma_start(
                        wcast(wd2_s[:]),
                        wcast(wd2_d.ap().rearrange("(c p) n -> p c n", p=128)))
                    bq1_s = decp.tile([1, F], F32, tag="bq1")
                    nc.sync.dma_start(wcast(bq1_s[:]), wcast(bq1_d.ap()))
                    bq2_s = decp.tile([1, 2 * F], F32, tag="bq2")
                    nc.sync.dma_start(wcast(bq2_s[:]), wcast(bq2_d.ap()))
                    for ct in range(K // 128):
                        d1ps = ps_mid.tile([128, F], F32, tag="mid")
                        nc.tensor.matmul(d1ps[:], mmcast(cbt_s[:, ts(ct, 128)]),
                                         mmcast(wd1_s[:]), start=True, stop=False)
                        bias_mm(d1ps, bq1_s, 0, F)
                        d1 = decp.tile([128, F], F32, tag="dec1")
                        nc.scalar.activation(wcast(d1[:]), d1ps[:], AF.Relu)
                        d1t = decp.tile([128, F], F32, tag="dec1t")
                        transpose_batch(d1t, d1, 4)
                        d2ps = ps_mid.tile([128, 2 * F], F32, tag="sc", bufs=1)
                        for j in range(2):
                            for c in range(4):
                                nc.tensor.matmul(
                                    d2ps[:, ts(j, F)],
                                    mmcast(d1t[:, ts(c, 128)]),
                                    mmcast(wd2_s[:, c, ts(j, F)]),
                                    start=(c == 0), stop=False)
                            bias_mm(d2ps, bq2_s, j * F, (j + 1) * F)
                        dsb = decp.tile([128, 2 * F], F32, tag="decout")
                        nc.scalar.activation(dsb[:], d2ps[:], AF.Copy)
                        st = nc.gpsimd.dma_start(dtab_d.ap()[ts(ct, 128), :], dsb[:])
                        dstores.append(st.ins)

            # ---- main loop over 128-row sub-blocks ----
            with tc.tile_pool(name="acts", bufs=acts_bufs) as acts:
                for s in range(nsub):
                    tf_t = acts.tile([128, T], F32, tag="tf")
                    nc.sync.dma_start(wcast(tf_t[:]), wcast(tf_d.ap()[ts(s, SUB), :]))
                    df_t = acts.tile([128, D], F32, tag="df")
                    nc.sync.dma_start(wcast(df_t[:]), wcast(df_d.ap()[ts(s, SUB), :]))

                    tfT = acts.tile([128, T], F32, tag="tfT")
                    transpose_batch(tfT, tf_t, T // 128)
                    dfT = acts.tile([D, 128], F32, tag="dfT")
                    transpose128(dfT[:], df_t[:], pdim=D)

                    # t = tf @ tp_w.T + tp_b
                    t_ps = ps_mid.tile([128, F], F32, tag="mid")
                    for c in range(T // 128):
                        nc.tensor.matmul(t_ps[:], mmcast(tfT[:, ts(c, 128)]),
                                         mmcast(wt_s[:, c, :]),
                                         start=(c == 0), stop=False)
                    bias_mm(t_ps, bt_s, 0, F)
                    t_sb = acts.tile([128, F], F32, tag="t")
                    nc.scalar.activation(t_sb[:], t_ps[:], AF.Copy)

                    # d = df @ dp_w.T + dp_b
                    d_ps = ps_mid.tile([128, F], F32, tag="mid")
                    nc.tensor.matmul(d_ps[:], mmcast(dfT[:]), mmcast(wd_s[:]),
                                     start=True, stop=False)
                    bias_mm(d_ps, bd_s, 0, F)
                    d_sb = acts.tile([128, F], F32, tag="d")
                    nc.scalar.activation(d_sb[:], d_ps[:], AF.Copy)

                    def layernorm(x_ap, tag):
                        # (x-mean)*rsqrt(var+eps); x_ap may live in PSUM
                        st = small.tile([128, 6], F32, tag="bnst")
                        nc.vector.bn_stats(st[:], x_ap)
                        mv = small.tile([128, 2], F32, tag="bnmv")
                        nc.vector.bn_aggr(mv[:], st[:])
                        sd = small.tile([128, 1], F32, tag="bnsd")
                        nc.scalar.activation(sd[:], mv[:, 1:2], AF.Sqrt,
                                             bias=eps_t[:])
                        a = small.tile([128, 1], F32, tag="bna")
                        nc.vector.reciprocal(a[:], sd[:])
                        o = acts.tile([128, F], F32, tag=tag)
                        nc.vector.tensor_scalar(wcast(o[:]), x_ap, mv[:, 0:1], a[:],
                                                op0=ALU.subtract, op1=ALU.mult)
                        return o

                    if stage <= 1:
                        continue
                    lnt = layernorm(t_ps[:], "lnt")
                    lnd = layernorm(d_ps[:], "lnd")

                    lntT = acts.tile([128, F], F32, tag="lntT")
                    transpose_batch(lntT, lnt, 4)
                    lndT = acts.tile([128, F], F32, tag="lndT")
                    transpose_batch(lndT, lnd, 4)

                    ad_ps = ps_mid.tile([128, F], F32, tag="mid")
                    for c in range(4):
                        nc.tensor.matmul(ad_ps[:], mmcast(lntT[:, ts(c, 128)]),
                                         mmcast(wa1_s[:, c, :]),
                                         start=(c == 0), stop=(c == 3))
                    at_ps = ps_mid.tile([128, F], F32, tag="mid")
                    for c in range(4):
                        nc.tensor.matmul(at_ps[:], mmcast(lndT[:, ts(c, 128)]),
                                         mmcast(wa2_s[:, c, :]),
                                         start=(c == 0), stop=(c == 3))

                    d2_sb = acts.tile([128, F], F32, tag="d2")
                    nc.vector.tensor_tensor(wcast(d2_sb[:]), d_sb[:], ad_ps[:],
                                            op=ALU.add)
                    t2_sb = acts.tile([128, F], F32, tag="t2")
                    nc.vector.tensor_tensor(wcast(t2_sb[:]), t_sb[:], at_ps[:],
                                            op=ALU.add)

                    d2T = acts.tile([128, F], F32, tag="d2T")
                    transpose_batch(d2T, d2_sb, 4)
                    t2T = acts.tile([128, F], F32, tag="t2T")
                    transpose_batch(t2T, t2_sb, 4)

                    if stage <= 2:
                        continue
                    u_ps = ps_mid.tile([128, F], F32, tag="mid")
                    for c in range(4):
                        nc.tensor.matmul(u_ps[:], mmcast(d2T[:, ts(c, 128)]),
                                         mmcast(we1d_s[:, c, :]),
                                         start=(c == 0), stop=False)
                    for c in range(4):
                        nc.tensor.matmul(u_ps[:], mmcast(t2T[:, ts(c, 128)]),
                                         mmcast(we1t_s[:, c, :]),
                                         start=False, stop=False)
                    bias_mm(u_ps, bu_s, 0, F)

                    x2 = layernorm(u_ps[:], "x2")
                    h1 = acts.tile([128, F], F32, tag="h1")
                    nc.scalar.activation(wcast(h1[:]), x2[:], AF.Relu)

                    h1T = acts.tile([128, F], F32, tag="h1T")
                    transpose_batch(h1T, h1, 4)

                    h_ps = ps_mid.tile([128, L], F32, tag="mid")
                    for c in range(4):
                        nc.tensor.matmul(h_ps[:], mmcast(h1T[:, ts(c, 128)]),
                                         mmcast(we2_s[:, c, :]),
                                         start=(c == 0), stop=False)
                    bias_mm(h_ps, bh_s, 0, L)
                    h_sb = acts.tile([128, L], F32, tag="h")
                    nc.scalar.activation(wcast(h_sb[:]), h_ps[:], AF.Copy)

                    # |h|^2 per row (reads PSUM; off the hT critical path)
                    sqh = small.tile([128, L], F32, tag="sqh")
                    h2c = small.tile([128, 1], F32, tag="h2c")
                    nc.scalar.activation(sqh[:], h_ps[:], AF.Square,
                                         accum_out=h2c[:])
                    nc.sync.dma_start(
                        h2_d.ap()[s, :].rearrange("(p a) -> p a", a=1), h2c[:])

                    if stage <= 3:
                        continue
                    # hT augmented with a ones row: one K=65 matmul adds -|c|^2
                    hT = acts.tile([L + 1, 128], F32, tag="hT")
                    transpose128(hT[:L, :], h_sb[:], pdim=L)
                    nc.scalar.activation(wcast(hT[L:L + 1, :]), ones1[:], AF.Copy)

                    # full-width scores in one 4-bank PSUM tile; direct
                    # Max8/MaxIndex over all 2048 codes
                    sc_ps = ps_mid.tile([128, K], F32, tag="sc", bufs=1)
                    for j in range(4):
                        nc.tensor.matmul(sc_ps[:, ts(j, F)], mmcast(hT[:]),
                                         mmcast(cbq_s[:, ts(j, F)]),
                                         start=True, stop=True)
                    if stage <= 4:
                        continue
                    m8v = small.tile([128, 8], F32, tag="m8v")
                    nc.vector.max(m8v[:], sc_ps[:])
                    m8i8 = small.tile([128, 8], U32, tag="m8i8")
                    nc.vector.max_index(m8i8[:], m8v[:], sc_ps[:])
                    idxu = m8i8
                    nc.sync.dma_start(m8v_d.ap()[s, :, :], m8v[:])
                    nc.sync.dma_start(
                        m8i_d.ap()[s, :].rearrange("(p a) -> p a", a=1),
                        idxu[:, 0:1])

                    if not with_gather:
                        continue
                    # rec gather: dtab[idx] -> rec rows (a tie can produce an
                    # out-of-range summed index; those rows are host-refined)
                    rg = recp.tile([128, 2 * F], F32, tag="rg")
                    g = nc.gpsimd.indirect_dma_start(
                        out=rg[:], out_offset=None,
                        in_=dtab_d.ap(),
                        in_offset=bass.IndirectOffsetOnAxis(ap=idxu[:, 0:1], axis=0),
                        bounds_check=K - 1, oob_is_err=False,
                    )
                    if with_deps:
                        for stn in dstores:
                            add_dep_helper(g.ins, stn,
                                           reason="gather after table build")
                    nc.sync.dma_start(rec_d.ap()[ts(s, SUB), :], rg[:])

    return nc


def _fold_weights(I):
    """Host-side weight folding in float64. Returns dict of device inputs."""
    f8 = {k: np.asarray(v, np.float64) for k, v in I.items()}
    wv = f8['in_w'][2 * F:]
    bv = f8['in_b'][2 * F:]
    ow = f8['out_w']
    ob = f8['out_b']
    Acore = (ow @ wv).T                      # [F_in, F_out]
    A1 = (f8['n1_w'][:, None] * Acore)
    A2 = (f8['n2_w'][:, None] * Acore)
    cA1 = f8['n1_b'] @ Acore + bv @ ow.T + ob
    cA2 = f8['n2_b'] @ Acore + bv @ ow.T + ob
    Ae = f8['e_w1'][:, :F].T                 # [F, F]
    Be = f8['e_w1'][:, F:].T
    bu = f8['e_b1'] + (f8['dp_b'] + cA1) @ Ae + (f8['tp_b'] + cA2) @ Be
    cb = f8['codebook']
    dev = {
        'wt': f8['tp_w'].T,
        'wd': f8['dp_w'].T, 'bd': f8['dp_b'][None, :],
        'wa1': A1, 'wa2': A2,
        'we1d': Ae, 'we1t': Be, 'we2': f8['e_w2'].T,
        'cbq': np.vstack([2.0 * cb.T, -(cb ** 2).sum(1)[None, :]]),
        'cbt': cb.T, 'wd1': f8['d_w1'].T, 'wd2': f8['d_w2'].T,
        'bt': f8['tp_b'][None, :],
        'bu': bu[None, :], 'bh': f8['e_b2'][None, :],
        'bq1': f8['d_b1'][None, :], 'bq2': f8['d_b2'][None, :],
        'lnw': f8['e_lnw'][None, :], 'lnb': f8['e_lnb'][None, :],
    }
    return {k: np.ascontiguousarray(v, dtype=np.float32) for k, v in dev.items()}


def _host_forward_rows(I, rows_idx):
    """Exact float64 recompute of h for selected global row indices."""
    f8 = {k: np.asarray(v, np.float64) for k, v in I.items()}
    df = f8['df'][rows_idx]
    tf = f8['tf'][rows_idx]

    def ln(x, w, b):
        m = x.mean(-1, keepdims=True)
        v = ((x - m) ** 2).mean(-1, keepdims=True)
        return (x - m) / np.sqrt(v + 1e-5) * w + b

    d = df @ f8['dp_w'].T + f8['dp_b']
    t = tf @ f8['tp_w'].T + f8['tp_b']
    wv = f8['in_w'][2 * F:]
    bv = f8['in_b'][2 * F:]
    ow = f8['out_w']
    ob = f8['out_b']
    Acore = (ow @ wv).T
    d2 = d + ln(t, f8['n1_w'], f8['n1_b']) @ Acore + (bv @ ow.T + ob)
    t2 = t + ln(d, f8['n2_w'], f8['n2_b']) @ Acore + (bv @ ow.T + ob)
    u = d2 @ f8['e_w1'][:, :F].T + t2 @ f8['e_w1'][:, F:].T + f8['e_b1']
    h1 = np.maximum(ln(u, f8['e_lnw'], f8['e_lnb']), 0.0)
    h = h1 @ f8['e_w2'].T + f8['e_b2']
    return h


_PROGRAM_CACHE = {}
LAST_RESULTS = None


def kernel(**inputs):
    import os
    from concourse import bass_utils

    if 'prog' not in _PROGRAM_CACHE:
        nc = build_program()
        nc.compile()
        _PROGRAM_CACHE['prog'] = nc
    nc = _PROGRAM_CACHE['prog']
    trace = os.environ.get("KERNEL_TRACE", "0") == "1"

    dev_w = _fold_weights(inputs)
    tf_full = np.ascontiguousarray(np.asarray(inputs['tf'], np.float32))
    df_full = np.ascontiguousarray(np.asarray(inputs['df'], np.float32))

    in_maps = []
    for c in range(NCORES):
        m = dict(dev_w)
        m['tf'] = tf_full[c * BC:(c + 1) * BC]
        m['df'] = df_full[c * BC:(c + 1) * BC]
        in_maps.append(m)

    res = bass_utils.run_bass_kernel_spmd(nc, in_maps, core_ids=list(range(NCORES)),
                                          trace=trace)
    global LAST_RESULTS
    LAST_RESULTS = res

    rec = np.empty((B, 2 * F), np.float32)
    m8v = np.empty((B, 8), np.float32)
    m8i = np.empty((B,), np.int64)
    h2 = np.empty((B,), np.float32)
    for c in range(NCORES):
        r = res.results[c]
        rec[c * BC:(c + 1) * BC] = r['rec']
        m8v[c * BC:(c + 1) * BC] = r['m8v'].reshape(BC, 8)
        m8i[c * BC:(c + 1) * BC] = r['m8i'].reshape(BC).astype(np.int64)
        h2[c * BC:(c + 1) * BC] = r['h2'].reshape(BC)

    idx = np.minimum(m8i, K - 1).astype(np.int32)
    s0 = m8v[:, 0].astype(np.float64)
    dist_min = h2.astype(np.float64) - s0

    # exact re-solve of near-tie rows (fp22 device noise ~1e-7 abs on scores)
    gap = (m8v[:, 0] - m8v[:, 1]).astype(np.float64)
    flagged = np.nonzero(gap < 2e-6)[0]
    if flagged.size:
        cb = np.asarray(inputs['codebook'], np.float64)
        hx = _host_forward_rows(inputs, flagged)
        sneg = 2.0 * hx @ cb.T - (cb ** 2).sum(1)
        new_idx = sneg.argmax(1)
        changed = flagged[new_idx != idx[flagged]]
        idx[flagged] = new_idx.astype(np.int32)
        dist_min[flagged] = (hx ** 2).sum(1) - sneg.max(1)
        if changed.size:
            f8 = {k: np.asarray(v, np.float64) for k, v in inputs.items()}
            q = f8['codebook'][idx[changed]]
            rec_fix = (np.maximum(q @ f8['d_w1'].T + f8['d_b1'], 0.0)
                       @ f8['d_w2'].T + f8['d_b2'])
            rec[changed] = rec_fix.astype(np.float32)

    loss = np.float32(1.25 * dist_min.mean() / L)
    return rec, loss, idx


if __name__ == "__main__":
    import reference
    ins = {k: np.asarray(v) for k, v in reference.setup_inputs().items()}
    out = kernel(**ins)
    print([getattr(o, 'shape', o) for o in out])
